# revision 10
# baseline (speedup 1.0000x reference)
"""MoE expert-pool kernel for Trainium2, 8 NeuronCores, expert-parallel.

Strategy:
  - Host: route tokens to experts (distinct (token,expert) pairs, combined
    routing weight per pair), gather per-expert token blocks, pad to a
    common capacity C, cast to bf16.
  - Device (per core = one expert): YT = W2^T @ gelu(W1^T @ XT + b1),
    all operands kept transposed so both weight matrices are used in their
    native layout as the stationary (lhsT) matmul operand. bf16 inputs,
    fp32 PSUM accumulation.
  - Host: scatter-add cw * (Y + b2) back to the [T, H] output.

Hardcoded problem shape: T=4096, H=1024, F=4096, E=8, K=2 (fp32 inputs).
"""

import sys
import types

import numpy as np
import ml_dtypes

H = 1024
F = 4096
E = 8
N_CORES = 8
PART = 128
TOK_CHUNK = 512  # fp32 PSUM bank = 512 columns


def _install_axon_trace_shim():
    """Make run_bass_kernel_spmd(trace=True) survive images that lack
    antenv.axon_hooks (tracing degrades gracefully if the hook .so is
    unavailable)."""
    try:
        import antenv.axon_hooks  # noqa: F401
        return
    except ImportError:
        pass
    mod = types.ModuleType("antenv.axon_hooks")
    mod._hook = None

    def set_axon_ntff_profile_hook(h):
        mod._hook = h

    def get_axon_ntff_profile_hook():
        return mod._hook

    mod.set_axon_ntff_profile_hook = set_axon_ntff_profile_hook
    mod.get_axon_ntff_profile_hook = get_axon_ntff_profile_hook
    sys.modules["antenv.axon_hooks"] = mod
    try:
        import antenv
        antenv.axon_hooks = mod
    except ImportError:
        pass
    try:
        from trn_agent_boot.trn_boot import _ntff_profile_via_ctypes
        mod._hook = _ntff_profile_via_ctypes("/opt/axon/libaxon_pjrt.so")
    except Exception:
        pass


_install_axon_trace_shim()

_PROGRAM_CACHE = {}


def _build_program(C):
    """Build + bacc-compile the per-core Bass program for capacity C."""
    import concourse.mybir as mybir
    import concourse.tile as tile
    from concourse import bacc

    bf16 = mybir.dt.bfloat16
    f32 = mybir.dt.float32

    KT1 = H // PART   # 8  k-tiles for mm1 (contract over H)
    MT1 = F // PART   # 32 m-tiles for mm1 (output partitions = F chunks)
    KT2 = F // PART   # 32 k-tiles for mm2 (contract over F)
    MT2 = H // PART   # 8  m-tiles for mm2 (output partitions = H chunks)

    # token chunks (PSUM free-dim limit 512 for fp32)
    chunks = []
    off = 0
    while off < C:
        n = min(TOK_CHUNK, C - off)
        chunks.append((off, n))
        off += n

    W1_MG = 512     # W1 dma column-group width
    W2_MG = 512     # W2 dma column-group width
    WARM_MMS = 82   # dummy matmuls to lift the HAM clock gate during DMA ramp

    nc = bacc.Bacc("TRN2", target_bir_lowering=False, debug=False,
                   num_devices=N_CORES)

    xt_d = nc.dram_tensor("xt", [H, C], bf16, kind="ExternalInput")
    w1_d = nc.dram_tensor("w1", [H, F], bf16, kind="ExternalInput")
    w2_d = nc.dram_tensor("w2", [F, H], bf16, kind="ExternalInput")
    b1_d = nc.dram_tensor("b1t", [PART, MT1], f32, kind="ExternalInput")
    yt_d = nc.dram_tensor("yt", [H, C], f32, kind="ExternalOutput")

    # DRAM views with the partition dim innermost: [p, k, cols]
    xt_v = xt_d.ap().rearrange("(k p) c -> p k c", p=PART)
    w1_v = w1_d.ap().rearrange("(k p) f -> p k f", p=PART)
    w2_v = w2_d.ap().rearrange("(k p) f -> p k f", p=PART)

    with tile.TileContext(nc) as tc:
        with (
            tc.tile_pool(name="big", bufs=1) as big_pool,
            tc.tile_pool(name="consts", bufs=1) as consts,
            tc.tile_pool(name="stage", bufs=4) as stage_pool,
            tc.tile_pool(name="psum", bufs=4, space="PSUM") as psum_pool,
            tc.tile_pool(name="wpsum", bufs=1, space="PSUM") as wpsum_pool,
        ):
            gelu = mybir.ActivationFunctionType.Gelu

            # PE pre-warm: zero-tile matmuls keep the PE busy through the
            # HAM activity window so the real stream starts at 2.4 GHz.
            warm_sb = consts.tile([PART, PART], bf16)
            nc.gpsimd.memset(warm_sb[:], 0.0)
            wps = wpsum_pool.tile([PART, PART], f32)
            for _ in range(WARM_MMS):
                nc.tensor.matmul(wps[:], warm_sb[:], warm_sb[:],
                                 start=True, stop=True)

            b1_sb = consts.tile([PART, MT1], f32)

            xt_sb = big_pool.tile([PART, KT1, C], bf16)
            w1_sb = big_pool.tile([PART, KT1, F], bf16)
            w2_sb = big_pool.tile([PART, KT2, H], bf16)
            h_sb = big_pool.tile([PART, MT1, TOK_CHUNK], bf16)

            # DMA order = consumption order. Critical prefix (gates the
            # first matmul group): chunk-0 tokens + W1's first m-tile,
            # issued on two queue engines to halve issue serialization.
            t00, tn0 = chunks[0]
            nc.sync.dma_start(xt_sb[:, :, t00:t00 + tn0],
                              xt_v[:, :, t00:t00 + tn0])
            nc.sync.dma_start(w1_sb[:, :, 0:PART], w1_v[:, :, 0:PART])
            nc.sync.dma_start(w1_sb[:, :, PART:W1_MG],
                              w1_v[:, :, PART:W1_MG])
            nc.sync.dma_start(b1_sb[:], b1_d.ap())
            for mg in range(W1_MG, F, W1_MG):
                nc.sync.dma_start(w1_sb[:, :, mg:mg + W1_MG],
                                  w1_v[:, :, mg:mg + W1_MG])
            for (t0, tn) in chunks[1:]:
                nc.sync.dma_start(xt_sb[:, :, t0:t0 + tn],
                                  xt_v[:, :, t0:t0 + tn])
            for mg in range(0, H, W2_MG):
                nc.sync.dma_start(w2_sb[:, :, mg:mg + W2_MG],
                                  w2_v[:, :, mg:mg + W2_MG])

            for (t0, tn) in chunks:
                # mm1 + gelu: h = gelu(W1^T X + b1) for this token chunk
                for m in range(MT1):
                    ps = psum_pool.tile([PART, TOK_CHUNK], f32, tag="ps",
                                        name="ps")
                    for k in range(KT1):
                        nc.tensor.matmul(
                            ps[:, :tn],
                            w1_sb[:, k, m * PART:(m + 1) * PART],
                            xt_sb[:, k, t0:t0 + tn],
                            start=(k == 0), stop=(k == KT1 - 1))
                    nc.scalar.activation(
                        h_sb[:, m, :tn], ps[:, :tn], gelu,
                        bias=b1_sb[:, m:m + 1], scale=1.0)

                # mm2: yt = W2^T h for this token chunk
                for m in range(MT2):
                    ps = psum_pool.tile([PART, TOK_CHUNK], f32, tag="ps",
                                        name="ps")
                    for k in range(KT2):
                        nc.tensor.matmul(
                            ps[:, :tn],
                            w2_sb[:, k, m * PART:(m + 1) * PART],
                            h_sb[:, k, :tn],
                            start=(k == 0), stop=(k == KT2 - 1))
                    out_sb = stage_pool.tile([PART, TOK_CHUNK], f32,
                                             tag="out", name="out")
                    last = (m == MT2 - 1) and (t0 + tn >= C)
                    if last:
                        # tail-critical: copy+DMA in halves so the first
                        # DMA overlaps the second copy
                        h0 = tn // 2
                        for (a, b) in ((0, h0), (h0, tn)):
                            nc.vector.tensor_copy(out_sb[:, a:b], ps[:, a:b])
                            nc.sync.dma_start(
                                yt_d.ap()[m * PART:(m + 1) * PART,
                                          t0 + a:t0 + b],
                                out_sb[:, a:b])
                    else:
                        nc.vector.tensor_copy(out_sb[:, :tn], ps[:, :tn])
                        nc.sync.dma_start(
                            yt_d.ap()[m * PART:(m + 1) * PART, t0:t0 + tn],
                            out_sb[:, :tn])

    nc.compile()
    return nc


def _route(expert_weights, selected_experts):
    """Distinct (token, expert) pairs with combined weights.

    Returns per-expert (token_ids, combined_weights)."""
    se = np.asarray(selected_experts).astype(np.int64)
    ew = np.asarray(expert_weights).astype(np.float32)
    routes = []
    for e in range(E):
        hit = (se == e)  # [T, K]
        tok = np.nonzero(hit.any(axis=1))[0]
        cw = (ew * hit).sum(axis=1)[tok]
        routes.append((tok, cw))
    return routes


def kernel(hidden_states, expert_weights, W1, b1, W2, b2, selected_experts):
    from concourse.bass_utils import run_bass_kernel_spmd

    hs = np.asarray(hidden_states)
    out_dtype = hs.dtype
    hs = hs.astype(np.float32)
    W1 = np.asarray(W1).astype(np.float32)
    b1 = np.asarray(b1).astype(np.float32)
    W2 = np.asarray(W2).astype(np.float32)
    b2 = np.asarray(b2).astype(np.float32)

    T = hs.shape[0]
    assert hs.shape[1] == H and W1.shape == (E, H, F) and W2.shape == (E, F, H)

    routes = _route(expert_weights, selected_experts)
    max_n = max(len(tok) for tok, _ in routes)
    C = max(PART, max_n)

    if C not in _PROGRAM_CACHE:
        _PROGRAM_CACHE[C] = _build_program(C)
    nc = _PROGRAM_CACHE[C]

    bf16 = ml_dtypes.bfloat16
    in_maps = []
    for e in range(E):
        tok, _ = routes[e]
        xt = np.zeros((H, C), dtype=bf16)
        if len(tok):
            xt[:, :len(tok)] = hs[tok].T.astype(bf16)
        in_maps.append({
            "xt": xt,
            "w1": W1[e].astype(bf16),
            "w2": W2[e].astype(bf16),
            "b1t": np.ascontiguousarray(b1[e].reshape(F // PART, PART).T),
        })

    res = run_bass_kernel_spmd(nc, in_maps, core_ids=list(range(N_CORES)))

    out = np.zeros((T, H), dtype=np.float32)
    for e in range(E):
        tok, cw = routes[e]
        if len(tok) == 0:
            continue
        yt = res.results[e]["yt"][:, :len(tok)].astype(np.float32)
        out[tok] += cw[:, None] * (yt.T + b2[e][None, :])
    return out.astype(out_dtype)


# revision 14
# speedup vs baseline: 1.0024x; 1.0024x over previous
"""MoE expert-pool kernel for Trainium2, 8 NeuronCores, expert-parallel.

Strategy:
  - Host: route tokens to experts (distinct (token,expert) pairs, combined
    routing weight per pair), gather per-expert token blocks, pad to a
    common capacity C, cast to bf16.
  - Device (per core = one expert): YT = W2^T @ gelu(W1^T @ XT + b1),
    all operands kept transposed so both weight matrices are used in their
    native layout as the stationary (lhsT) matmul operand. bf16 inputs,
    fp32 PSUM accumulation.
  - Host: scatter-add cw * (Y + b2) back to the [T, H] output.

Hardcoded problem shape: T=4096, H=1024, F=4096, E=8, K=2 (fp32 inputs).
"""

import sys
import types

import numpy as np
import ml_dtypes

H = 1024
F = 4096
E = 8
N_CORES = 8
PART = 128
TOK_CHUNK = 512  # fp32 PSUM bank = 512 columns


def _install_axon_trace_shim():
    """Make run_bass_kernel_spmd(trace=True) survive images that lack
    antenv.axon_hooks (tracing degrades gracefully if the hook .so is
    unavailable)."""
    try:
        import antenv.axon_hooks  # noqa: F401
        return
    except ImportError:
        pass
    mod = types.ModuleType("antenv.axon_hooks")
    mod._hook = None

    def set_axon_ntff_profile_hook(h):
        mod._hook = h

    def get_axon_ntff_profile_hook():
        return mod._hook

    mod.set_axon_ntff_profile_hook = set_axon_ntff_profile_hook
    mod.get_axon_ntff_profile_hook = get_axon_ntff_profile_hook
    sys.modules["antenv.axon_hooks"] = mod
    try:
        import antenv
        antenv.axon_hooks = mod
    except ImportError:
        pass
    try:
        from trn_agent_boot.trn_boot import _ntff_profile_via_ctypes
        mod._hook = _ntff_profile_via_ctypes("/opt/axon/libaxon_pjrt.so")
    except Exception:
        pass


_install_axon_trace_shim()

_PROGRAM_CACHE = {}


def _build_program(C):
    """Build + bacc-compile the per-core Bass program for capacity C."""
    import concourse.mybir as mybir
    import concourse.tile as tile
    from concourse import bacc

    bf16 = mybir.dt.bfloat16
    f32 = mybir.dt.float32

    KT1 = H // PART   # 8  k-tiles for mm1 (contract over H)
    MT1 = F // PART   # 32 m-tiles for mm1 (output partitions = F chunks)
    KT2 = F // PART   # 32 k-tiles for mm2 (contract over F)
    MT2 = H // PART   # 8  m-tiles for mm2 (output partitions = H chunks)

    # token chunks (PSUM free-dim limit 512 for fp32)
    chunks = []
    off = 0
    while off < C:
        n = min(TOK_CHUNK, C - off)
        chunks.append((off, n))
        off += n

    W1_MG = 512     # W1 dma column-group width
    W2_MG = 512     # W2 dma column-group width
    WARM_MMS = 82   # dummy matmuls to lift the HAM clock gate during DMA ramp

    nc = bacc.Bacc("TRN2", target_bir_lowering=False, debug=False,
                   num_devices=N_CORES)

    # xt is host-arranged chunk-major ([p][k][tok] per chunk, concatenated)
    # so every chunk's DMA reads fully-contiguous per-partition lines.
    xt_d = nc.dram_tensor("xt", [PART, KT1 * C], bf16, kind="ExternalInput")
    # w1h duplicates W1's first m-tile in SBUF layout for a contiguous
    # critical-path transfer.
    w1h_d = nc.dram_tensor("w1h", [PART, KT1 * PART], bf16,
                           kind="ExternalInput")
    w1_d = nc.dram_tensor("w1", [H, F], bf16, kind="ExternalInput")
    w2_d = nc.dram_tensor("w2", [F, H], bf16, kind="ExternalInput")
    b1_d = nc.dram_tensor("b1t", [PART, MT1], f32, kind="ExternalInput")
    yt_d = nc.dram_tensor("yt", [H, C], f32, kind="ExternalOutput")

    # DRAM views with the partition dim innermost: [p, k, cols]
    w1_v = w1_d.ap().rearrange("(k p) f -> p k f", p=PART)
    w2_v = w2_d.ap().rearrange("(k p) f -> p k f", p=PART)

    with tile.TileContext(nc) as tc:
        with (
            tc.tile_pool(name="big", bufs=1) as big_pool,
            tc.tile_pool(name="consts", bufs=1) as consts,
            tc.tile_pool(name="stage", bufs=4) as stage_pool,
            tc.tile_pool(name="psum", bufs=4, space="PSUM") as psum_pool,
            tc.tile_pool(name="wpsum", bufs=1, space="PSUM") as wpsum_pool,
        ):
            gelu = mybir.ActivationFunctionType.Gelu

            # PE pre-warm: zero-tile matmuls keep the PE busy through the
            # HAM activity window so the real stream starts at 2.4 GHz.
            warm_sb = consts.tile([PART, PART], bf16)
            nc.vector.memset(warm_sb[:], 0.0)
            wps = wpsum_pool.tile([PART, PART], f32)
            for _ in range(WARM_MMS):
                nc.tensor.matmul(wps[:], warm_sb[:], warm_sb[:],
                                 start=True, stop=True)

            b1_sb = consts.tile([PART, MT1], f32)

            xt_sb = big_pool.tile([PART, KT1, C], bf16)
            w1_sb = big_pool.tile([PART, KT1, F], bf16)
            w2_sb = big_pool.tile([PART, KT2, H], bf16)
            h_sb = big_pool.tile([PART, MT1, TOK_CHUNK], bf16)

            # DMA order = consumption order. Critical prefix (gates the
            # first matmul group): chunk-0 tokens + W1's first m-tile,
            # both fully contiguous in DRAM.
            def xt_src(t0, tn):
                return xt_d.ap()[:, t0 * KT1:(t0 + tn) * KT1].rearrange(
                    "p (k c) -> p k c", k=KT1)

            t00, tn0 = chunks[0]
            nc.sync.dma_start(xt_sb[:, :, t00:t00 + tn0], xt_src(t00, tn0))
            nc.sync.dma_start(
                w1_sb[:, :, 0:PART],
                w1h_d.ap().rearrange("p (k c) -> p k c", k=KT1))
            nc.sync.dma_start(w1_sb[:, :, PART:W1_MG],
                              w1_v[:, :, PART:W1_MG])
            nc.sync.dma_start(b1_sb[:], b1_d.ap())
            for mg in range(W1_MG, F, W1_MG):
                nc.sync.dma_start(w1_sb[:, :, mg:mg + W1_MG],
                                  w1_v[:, :, mg:mg + W1_MG])
            for (t0, tn) in chunks[1:]:
                nc.sync.dma_start(xt_sb[:, :, t0:t0 + tn], xt_src(t0, tn))
            for mg in range(0, H, W2_MG):
                nc.sync.dma_start(w2_sb[:, :, mg:mg + W2_MG],
                                  w2_v[:, :, mg:mg + W2_MG])

            for (t0, tn) in chunks:
                # mm1 + gelu: h = gelu(W1^T X + b1) for this token chunk
                for m in range(MT1):
                    ps = psum_pool.tile([PART, TOK_CHUNK], f32, tag="ps",
                                        name="ps")
                    for k in range(KT1):
                        nc.tensor.matmul(
                            ps[:, :tn],
                            w1_sb[:, k, m * PART:(m + 1) * PART],
                            xt_sb[:, k, t0:t0 + tn],
                            start=(k == 0), stop=(k == KT1 - 1))
                    nc.scalar.activation(
                        h_sb[:, m, :tn], ps[:, :tn], gelu,
                        bias=b1_sb[:, m:m + 1], scale=1.0)

                # mm2: yt = W2^T h for this token chunk
                for m in range(MT2):
                    ps = psum_pool.tile([PART, TOK_CHUNK], f32, tag="ps",
                                        name="ps")
                    for k in range(KT2):
                        nc.tensor.matmul(
                            ps[:, :tn],
                            w2_sb[:, k, m * PART:(m + 1) * PART],
                            h_sb[:, k, :tn],
                            start=(k == 0), stop=(k == KT2 - 1))
                    out_sb = stage_pool.tile([PART, TOK_CHUNK], f32,
                                             tag="out", name="out")
                    last = (m == MT2 - 1) and (t0 + tn >= C)
                    if last:
                        # tail-critical: copy+DMA in halves so the first
                        # DMA overlaps the second copy
                        h0 = tn // 2
                        for (a, b) in ((0, h0), (h0, tn)):
                            nc.vector.tensor_copy(out_sb[:, a:b], ps[:, a:b])
                            nc.sync.dma_start(
                                yt_d.ap()[m * PART:(m + 1) * PART,
                                          t0 + a:t0 + b],
                                out_sb[:, a:b])
                    else:
                        nc.vector.tensor_copy(out_sb[:, :tn], ps[:, :tn])
                        nc.sync.dma_start(
                            yt_d.ap()[m * PART:(m + 1) * PART, t0:t0 + tn],
                            out_sb[:, :tn])

    nc.compile()
    return nc


def _route(expert_weights, selected_experts):
    """Distinct (token, expert) pairs with combined weights.

    Returns per-expert (token_ids, combined_weights)."""
    se = np.asarray(selected_experts).astype(np.int64)
    ew = np.asarray(expert_weights).astype(np.float32)
    routes = []
    for e in range(E):
        hit = (se == e)  # [T, K]
        tok = np.nonzero(hit.any(axis=1))[0]
        cw = (ew * hit).sum(axis=1)[tok]
        routes.append((tok, cw))
    return routes


def kernel(hidden_states, expert_weights, W1, b1, W2, b2, selected_experts):
    from concourse.bass_utils import run_bass_kernel_spmd

    hs = np.asarray(hidden_states)
    out_dtype = hs.dtype
    hs = hs.astype(np.float32)
    W1 = np.asarray(W1).astype(np.float32)
    b1 = np.asarray(b1).astype(np.float32)
    W2 = np.asarray(W2).astype(np.float32)
    b2 = np.asarray(b2).astype(np.float32)

    T = hs.shape[0]
    assert hs.shape[1] == H and W1.shape == (E, H, F) and W2.shape == (E, F, H)

    routes = _route(expert_weights, selected_experts)
    max_n = max(len(tok) for tok, _ in routes)
    C = max(PART, max_n)

    if C not in _PROGRAM_CACHE:
        _PROGRAM_CACHE[C] = _build_program(C)
    nc = _PROGRAM_CACHE[C]

    bf16 = ml_dtypes.bfloat16
    KT1 = H // PART

    # token chunks must mirror _build_program's chunking
    chunk_sizes = []
    off = 0
    while off < C:
        n = min(TOK_CHUNK, C - off)
        chunk_sizes.append(n)
        off += n

    in_maps = []
    for e in range(E):
        tok, _ = routes[e]
        xt = np.zeros((H, C), dtype=bf16)
        if len(tok):
            xt[:, :len(tok)] = hs[tok].T.astype(bf16)
        # chunk-major SBUF layout: per chunk [p][k][tok], concatenated
        xt3 = xt.reshape(KT1, PART, C)
        parts = []
        t0 = 0
        for tn in chunk_sizes:
            parts.append(xt3[:, :, t0:t0 + tn].transpose(1, 0, 2).reshape(
                PART, KT1 * tn))
            t0 += tn
        xt_host = np.ascontiguousarray(np.concatenate(parts, axis=1))

        w1e = W1[e].astype(bf16)
        w1h = np.ascontiguousarray(
            w1e[:, :PART].reshape(KT1, PART, PART).transpose(1, 0, 2).reshape(
                PART, KT1 * PART))
        in_maps.append({
            "xt": xt_host,
            "w1h": w1h,
            "w1": w1e,
            "w2": W2[e].astype(bf16),
            "b1t": np.ascontiguousarray(b1[e].reshape(F // PART, PART).T),
        })

    res = run_bass_kernel_spmd(nc, in_maps, core_ids=list(range(N_CORES)))

    out = np.zeros((T, H), dtype=np.float32)
    for e in range(E):
        tok, cw = routes[e]
        if len(tok) == 0:
            continue
        yt = res.results[e]["yt"][:, :len(tok)].astype(np.float32)
        out[tok] += cw[:, None] * (yt.T + b2[e][None, :])
    return out.astype(out_dtype)
